# revision 20
# baseline (speedup 1.0000x reference)
"""Block-causal attention kernel for trn2, sharded over 8 NeuronCores.

Sharding: device d handles batch b = d // 4 and heads hA = 2*(d%4),
hB = hA + 1.  Each device computes its two heads' attention plus its
partial output projection partialT[c, t] = sum_h yT_h @ Wo_h; the host
sums the 4 partials per batch and adds bo.

All matmuls run in float32r (TF32-like, full PE rate at N>=256).
Softmax is computed without max-subtraction (scores are ~N(0,1); exp is
safe in fp32) with the denominator produced by a ones-column in the AV
stationary operand.
"""

import json

import numpy as np

import concourse.bass as bass
import concourse.mybir as mybir
import concourse.tile as tile
from concourse.bass_utils import run_bass_kernel_spmd
from concourse.masks import make_identity
from concourse.vector_clock import ScopedClock

F32R = mybir.dt.float32r
F32 = mybir.dt.float32

VP, B, C, H, W = 8, 2, 512, 16, 16
NH = 8
HD = C // NH  # 64
HWD = H * W  # 256 = block size
T = VP * HWD  # 2048
NCORES = 8
HEADS_PER_CORE = 2
SCALE = 1.0 / np.sqrt(HD)

# ---------------------------------------------------------------------------
# Container workarounds (walrus in this image rejects >1 sync wait/update per
# instruction; Tile's tail drain carries many).
# ---------------------------------------------------------------------------


def _split_syncs(bir_bytes: bytes) -> bytes:
    j = json.loads(bir_bytes)
    changed = False
    for fn in j.get("functions", []):
        for bb in fn.get("blocks", []):
            out = []
            for inst in bb.get("instructions", []):
                si = inst.get("sync_info")
                if not si:
                    out.append(inst)
                    continue
                waits = si.get("on_wait") or []
                upds = si.get("on_update") or []
                if len(waits) > 1:
                    for i, w in enumerate(waits[:-1]):
                        out.append(
                            {
                                "debug": inst.get("debug", 0),
                                "engine": inst["engine"],
                                "ins": [],
                                "name": f"{inst['name']}_sw{i}",
                                "opcode": "EventSemaphore",
                                "outs": [],
                                "sync_info": {"on_update": [], "on_wait": [w]},
                            }
                        )
                    si["on_wait"] = waits[-1:]
                    changed = True
                out.append(inst)
                if len(upds) > 1:
                    si["on_update"] = upds[:1]
                    for i, u in enumerate(upds[1:]):
                        out.append(
                            {
                                "debug": inst.get("debug", 0),
                                "engine": inst["engine"],
                                "ins": [],
                                "name": f"{inst['name']}_su{i}",
                                "opcode": "EventSemaphore",
                                "outs": [],
                                "sync_info": {"on_update": [u], "on_wait": []},
                            }
                        )
                    changed = True
            bb["instructions"] = out
    return json.dumps(j).encode() if changed else bir_bytes


_patched = False


def _install_patches():
    global _patched
    if _patched:
        return
    _patched = True

    import concourse.bass2jax as bass2jax
    from concourse.bass_utils import compile_bir_kernel as _real_compile

    def patched_compile(bir_json, tmpdir, neff_name="file.neff"):
        return _real_compile(_split_syncs(bir_json), tmpdir, neff_name=neff_name)

    bass2jax.compile_bir_kernel = patched_compile

    def _drain_and_barrier(self, tick_clock, wait_clock):
        nc = self.nc
        drain_inst = nc.sync.drain()
        wait_clock.add_sem_waits(
            drain_inst.ins, ScopedClock({None: tick_clock.global_clock})
        )
        si = drain_inst.ins.sync_info
        waits = list(si.on_wait or [])
        if len(waits) > 1:
            si.on_wait = waits[:1]
            for w in waits[1:]:
                d2 = nc.sync.drain()
                d2.ins.sync_info = mybir.SyncInfo(on_wait=[w], on_update=[])
        nc.all_engine_barrier()
        assert self.sems is not None
        popped = nc._tile_sem_poison_stack.pop()
        assert popped is self._sem_poison
        nc.clear_and_free_semaphores(list(self.sems.allocated().values()))
        nc.all_engine_barrier()

    tile.TileContext._drain_and_barrier = _drain_and_barrier


# ---------------------------------------------------------------------------
# Device program (SPMD — same program on all 8 cores, different data)
# ---------------------------------------------------------------------------


def _build_program():
    _install_patches()
    nc = bass.Bass("TRN2", target_bir_lowering=False, debug=False, num_devices=NCORES)

    xT = nc.dram_tensor("xT", [C, T], F32R, kind="ExternalInput")
    wq = nc.dram_tensor("wq", [C, 128], F32R, kind="ExternalInput")
    wk = nc.dram_tensor("wk", [C, 128], F32R, kind="ExternalInput")
    wv = nc.dram_tensor("wv", [C, 128], F32R, kind="ExternalInput")
    # wo[0:64, s, :] = Wo rows of head A (s=0) / head B (s=1)
    wo = nc.dram_tensor("wo", [64, 2, C], F32R, kind="ExternalInput")
    partialT = nc.dram_tensor("partialT", [C, T], F32, kind="ExternalOutput")

    import os

    debug = bool(os.environ.get("KDEBUG"))
    dbg = {}
    if debug:
        dbg["qT"] = nc.dram_tensor("dbg_qT", [128, T], F32, kind="ExternalOutput")
        dbg["kT"] = nc.dram_tensor("dbg_kT", [128, T], F32, kind="ExternalOutput")
        dbg["vA"] = nc.dram_tensor("dbg_vA", [128, 16, 65], F32, kind="ExternalOutput")
        dbg["pA1"] = nc.dram_tensor("dbg_pA1", [128, 1024], F32, kind="ExternalOutput")
        dbg["yA"] = nc.dram_tensor("dbg_yA", [65, 2048], F32, kind="ExternalOutput")
        dbg["rbcA"] = nc.dram_tensor("dbg_rbcA", [64, 2048], F32, kind="ExternalOutput")
        dbg["yTA"] = nc.dram_tensor("dbg_yTA", [64, T], F32, kind="ExternalOutput")

    NB = T // 256  # 8 blocks of 256 (= one VP slice each)

    with tile.TileContext(nc) as tc:
        with (
            tc.tile_pool(name="persist", bufs=1) as pers,
            tc.tile_pool(name="work", bufs=3) as work,
            tc.tile_pool(name="ppool", bufs=3) as ppool,
            tc.tile_pool(name="ypool", bufs=2) as ypool,
            tc.tile_pool(name="dpool", bufs=2, space="DRAM") as dpool,
        ):
            # ---- persistent SBUF tiles
            qT_t = pers.tile([128, T], F32R)  # rows 0-63 head A, 64-127 head B
            kT_t = pers.tile([128, T], F32R)
            vA_t = pers.tile([128, NB * 2, 65], F32R)  # [k-tok, chunk, v|1]
            vB_t = pers.tile([128, NB * 2, 65], F32R)
            yTA_t = pers.tile([64, T], F32R)
            yTB_t = pers.tile([64, T], F32R)
            wq_t = pers.tile([128, 4, 128], F32R)
            wk_t = pers.tile([128, 4, 128], F32R)
            wv_t = pers.tile([128, 4, 128], F32R)
            wo_t = pers.tile([64, 2, C], F32R)
            ident = pers.tile([128, 128], F32)
            ones_t = pers.tile([128, 1], F32)
            onesr_t = pers.tile([128, 64], F32R)
            xT_t = pers.tile([128, 4, T], F32R)

            nc.gpsimd.dma_start(out=wq_t[:], in_=wq.rearrange("(c p) m -> p c m", p=128))
            nc.gpsimd.dma_start(out=wk_t[:], in_=wk.rearrange("(c p) m -> p c m", p=128))
            nc.gpsimd.dma_start(out=wv_t[:], in_=wv.rearrange("(c p) m -> p c m", p=128))
            nc.gpsimd.dma_start(out=wo_t[:], in_=wo[:])
            make_identity(nc, ident)
            nc.vector.memset(ones_t[:], 1.0)
            nc.vector.tensor_copy(onesr_t[:], ones_t[:].to_broadcast([128, 64]))
            # preload the exp table set while the x DMA streams in
            warm = work.tile([128, 1], F32, tag="warm")
            nc.scalar.activation(
                warm[:], ones_t[:], mybir.ActivationFunctionType.Exp
            )
            for cc in range(4):
                for hh in range(4):
                    hsl = slice(hh * (T // 4), (hh + 1) * (T // 4))
                    eng = nc.sync if (cc * 4 + hh) % 2 == 0 else nc.gpsimd
                    eng.dma_start(
                        out=xT_t[:, cc, hsl], in_=xT[cc * 128 : (cc + 1) * 128, hsl]
                    )

            # ones columns of v' (free-dim broadcast copy, rounds to f32r)
            nc.vector.tensor_copy(
                vA_t[:, :, 64:65], ones_t[:].to_broadcast([128, NB * 2, 1])
            )
            nc.vector.tensor_copy(
                vB_t[:, :, 64:65], ones_t[:].to_broadcast([128, NB * 2, 1])
            )

            # ---- phase 1: qkv projections
            with tc.tile_pool(name="ps1", bufs=2, space="PSUM") as ps1:
                for sp in range(4):  # 512-wide token spans
                    sl = slice(sp * 512, (sp + 1) * 512)
                    psq = ps1.tile([128, 512], F32, tag="q")
                    psk = ps1.tile([128, 512], F32, tag="k")
                    for cc in range(4):
                        nc.tensor.matmul(
                            psq[:],
                            wq_t[:, cc, :],
                            xT_t[:, cc, sl],
                            start=(cc == 0),
                            stop=(cc == 3),
                        )
                    for cc in range(4):
                        nc.tensor.matmul(
                            psk[:],
                            wk_t[:, cc, :],
                            xT_t[:, cc, sl],
                            start=(cc == 0),
                            stop=(cc == 3),
                        )
                    nc.vector.tensor_copy(qT_t[:, sl], psq[:])
                    nc.vector.tensor_copy(kT_t[:, sl], psk[:])

                for sp in range(4):
                    sl = slice(sp * 512, (sp + 1) * 512)
                    psv = ps1.tile([128, 512], F32, tag="v")
                    for cc in range(4):
                        nc.tensor.matmul(
                            psv[:],
                            wv_t[:, cc, :],
                            xT_t[:, cc, sl],
                            start=(cc == 0),
                            stop=(cc == 3),
                        )
                    vT_sb = work.tile([128, 512], F32, tag="vT")
                    nc.vector.tensor_copy(vT_sb[:], psv[:])
                    for i in range(4):
                        j = sp * 4 + i  # k-chunk index (128 tokens)
                        trp = ps1.tile([128, 128], F32, tag="tr")
                        nc.tensor.transpose(
                            trp[:], vT_sb[:, i * 128 : (i + 1) * 128], ident[:]
                        )
                        nc.vector.tensor_copy(vA_t[:, j, 0:64], trp[:, 0:64])
                        nc.vector.tensor_copy(vB_t[:, j, 0:64], trp[:, 64:128])

            if debug:
                qT_f = work.tile([128, T], F32, tag="dbgbig")
                nc.vector.tensor_copy(qT_f[:], qT_t[:].bitcast(F32))
                nc.sync.dma_start(out=dbg["qT"][:], in_=qT_f[:])
                kT_f = work.tile([128, T], F32, tag="dbgbig")
                nc.vector.tensor_copy(kT_f[:], kT_t[:].bitcast(F32))
                nc.sync.dma_start(out=dbg["kT"][:], in_=kT_f[:])
                vA_f = work.tile([128, 16 * 65], F32, tag="dbgbig")
                nc.vector.tensor_copy(
                    vA_f[:].rearrange("p (j m) -> p j m", j=16), vA_t[:].bitcast(F32)
                )
                nc.sync.dma_start(
                    out=dbg["vA"][:],
                    in_=vA_f[:].rearrange("p (j m) -> p j m", j=16),
                )

            # ---- phase 2+3: attention passes with fused output projection
            def normalize_span(p, spl, yA_ps, yB_ps, last=False):
                """Normalize absolute 512-span 2*p+spl into yT tiles."""
                sp = 2 * p + spl
                sl = slice(sp * 512, (sp + 1) * 512)
                psl = slice(spl * 512, (spl + 1) * 512)
                for hi, (y_ps, yT_t) in enumerate(((yA_ps, yTA_t), (yB_ps, yTB_t))):
                    rd = work.tile([128, 512], F32, tag="rd")
                    nc.vector.reciprocal(rd[64:65, :], y_ps[64:65, psl])
                    rbc = work.tile([64, 512], F32, tag="rbc")
                    if last:
                        # tail path: broadcast via PE (ones ⊗ recipD) — avoids
                        # the DRAM round-trip latency on the critical tail
                        rdr = work.tile([128, 512], F32R, tag="rdr")
                        nc.vector.tensor_copy(rdr[64:65, :], rd[64:65, :])
                        bcp = pops.tile([128, 512], F32, tag="po")
                        nc.tensor.matmul(
                            bcp[0:64, :],
                            onesr_t[64:65, :],
                            rdr[64:65, :],
                            start=True,
                            stop=True,
                        )
                        nc.vector.tensor_copy(rbc[:], bcp[0:64, :])
                    else:
                        dscr = dpool.tile([1, 512], F32, tag="dscr")
                        nc.sync.dma_start(out=dscr[:], in_=rd[64:65, :])
                        nc.sync.dma_start(
                            out=rbc[:], in_=dscr[0:1, :].to_broadcast([64, 512])
                        )
                    nc.vector.tensor_mul(yT_t[:, sl], y_ps[0:64, psl], rbc[:])
                    if debug and hi == 0:
                        nc.sync.dma_start(out=dbg["rbcA"][:, sl], in_=rbc[:])
                        ydbg = work.tile([65, 512], F32, tag="ydbg")
                        nc.vector.tensor_copy(ydbg[:], y_ps[:, psl])
                        nc.sync.dma_start(out=dbg["yA"][:, sl], in_=ydbg[:])

            def project_span(p, spl):
                """Output-project absolute 512-span 2*p+spl (reads yT tiles)."""
                sp = 2 * p + spl
                sl = slice(sp * 512, (sp + 1) * 512)
                stage = work.tile([128, 4, 512], F32, tag="po_sb")
                for cc in range(4):
                    po = pops.tile([128, 512], F32, tag="po")
                    nc.tensor.matmul(
                        po[:],
                        wo_t[:, 0, cc * 128 : (cc + 1) * 128],
                        yTA_t[:, sl],
                        start=True,
                        stop=False,
                    )
                    nc.tensor.matmul(
                        po[:],
                        wo_t[:, 1, cc * 128 : (cc + 1) * 128],
                        yTB_t[:, sl],
                        start=False,
                        stop=True,
                    )
                    nc.vector.tensor_copy(stage[:, cc, :], po[:])
                    eng = nc.sync if cc % 2 == 0 else nc.gpsimd
                    eng.dma_start(
                        out=partialT[cc * 128 : (cc + 1) * 128, sl],
                        in_=stage[:, cc, :],
                    )

            with (
                tc.tile_pool(name="stps", bufs=1, space="PSUM") as stps,
                tc.tile_pool(name="yps", bufs=1, space="PSUM") as yps,
                tc.tile_pool(name="pops", bufs=2, space="PSUM") as pops,
            ):
                ytiles = {}
                # deferred normalize/outproj: normalize shortly after a span's
                # accumulation completes, project a couple of k-chunks later so
                # the recipD DRAM bounce hides behind queued PE work
                defer_norm = {(0, 4): (0, 0), (1, 0): (0, 1), (1, 12): (1, 0)}
                defer_proj = {(0, 6): (0, 0), (1, 2): (0, 1), (1, 14): (1, 0)}
                for p in range(2):
                    q0p = p * 1024
                    yA_ps = yps.tile([65, 1024], F32, tag="yA")
                    yB_ps = yps.tile([65, 1024], F32, tag="yB")
                    ytiles[p] = (yA_ps, yB_ps)
                    njc = 8 if p == 0 else 16
                    for j in range(njc):
                        jb = j // 2
                        qlo = max(256 * jb, q0p)
                        qend = q0p + 1024
                        segs = []
                        c = qlo
                        while c < qend:
                            nxt = min(qend, (c // 512 + 1) * 512)
                            segs.append((c, nxt))
                            c = nxt
                        ksl = slice(j * 128, (j + 1) * 128)
                        for g0, g1 in segs:
                            Lg = g1 - g0
                            # both heads packed in one 2-bank tile: A at
                            # [0, Lg), B at [Lg, 2Lg) -> one exp for both
                            stAB = stps.tile([128, 1024], F32, tag="st")
                            for s in range(Lg // 256):
                                qsl = slice(g0 + s * 256, g0 + (s + 1) * 256)
                                oa = slice(s * 256, (s + 1) * 256)
                                ob = slice(Lg + s * 256, Lg + (s + 1) * 256)
                                nc.tensor.matmul(
                                    stAB[:, oa],
                                    kT_t[0:64, ksl],
                                    qT_t[0:64, qsl],
                                    start=True,
                                    stop=True,
                                )
                                nc.tensor.matmul(
                                    stAB[:, ob],
                                    kT_t[64:128, ksl],
                                    qT_t[64:128, qsl],
                                    start=True,
                                    stop=True,
                                )
                            pAB = ppool.tile([128, 1024], F32R, tag="pAB")
                            nc.scalar.activation(
                                pAB[:, 0 : 2 * Lg],
                                stAB[:, 0 : 2 * Lg],
                                mybir.ActivationFunctionType.Exp,
                            )
                            if debug and p == 0 and j == 1:
                                pA_f = work.tile([128, 512], F32, tag="pAf")
                                nc.vector.tensor_copy(
                                    pA_f[:, 0:Lg], pAB[:, 0:Lg].bitcast(F32)
                                )
                                nc.sync.dma_start(
                                    out=dbg["pA1"][:, g0 - q0p : g1 - q0p],
                                    in_=pA_f[:, 0:Lg],
                                )
                            for s in range(Lg // 256):
                                qabs = g0 + s * 256
                                qrel = qabs - q0p
                                ysl = slice(qrel, qrel + 256)
                                jlast = 2 * (qabs // 256) + 1
                                # start=True clears has_written for the whole
                                # PSUM bank: only the first write per bank sets it
                                st = j == 0 and (qrel % 512) == 0
                                nc.tensor.matmul(
                                    yA_ps[:, ysl],
                                    vA_t[:, j, :],
                                    pAB[:, s * 256 : (s + 1) * 256],
                                    start=st,
                                    stop=(j == min(jlast, njc - 1)),
                                )
                                nc.tensor.matmul(
                                    yB_ps[:, ysl],
                                    vB_t[:, j, :],
                                    pAB[:, Lg + s * 256 : Lg + (s + 1) * 256],
                                    start=st,
                                    stop=(j == min(jlast, njc - 1)),
                                )
                        if (p, j) in defer_norm:
                            ps_, spl_ = defer_norm[(p, j)]
                            normalize_span(ps_, spl_, *ytiles[ps_])
                        if (p, j) in defer_proj:
                            ps_, spl_ = defer_proj[(p, j)]
                            project_span(ps_, spl_)
                normalize_span(1, 1, *ytiles[1])
                project_span(1, 1)
                if debug:
                    yTA_f = work.tile([64, T], F32, tag="dbgy")
                    nc.vector.tensor_copy(yTA_f[:], yTA_t[:].bitcast(F32))
                    nc.sync.dma_start(out=dbg["yTA"][:], in_=yTA_f[:])
    return nc


_NC_CACHE = None


def _get_program():
    global _NC_CACHE
    if _NC_CACHE is None:
        _NC_CACHE = _build_program()
    return _NC_CACHE


def kernel(x, Wqkv, bqkv, bo=None, Wo=None, **kw):
    # accept arbitrary kw order; reference signature: x, Wqkv, bqkv, Wo, bo
    if Wo is None:
        Wo = kw["Wo"]
    if bo is None:
        bo = kw["bo"]
    x = np.asarray(x, dtype=np.float32)
    Wqkv = np.asarray(Wqkv, dtype=np.float32)
    bqkv = np.asarray(bqkv, dtype=np.float32)
    Wo = np.asarray(Wo, dtype=np.float32)
    bo = np.asarray(bo, dtype=np.float32)
    assert np.all(bqkv == 0.0), "nonzero bqkv not supported by this kernel build"

    nc = _get_program()
    in_maps = []
    for d in range(NCORES):
        b = d // 4
        hA = HEADS_PER_CORE * (d % 4)
        hB = hA + 1
        # xT [C, T]: t = (v, h, w)
        xT = np.ascontiguousarray(
            x[:, b].transpose(1, 0, 2, 3).reshape(C, T)
        )
        qcols = np.r_[hA * HD : (hA + 1) * HD, hB * HD : (hB + 1) * HD]
        wq = np.ascontiguousarray(Wqkv[:, qcols] * SCALE)
        wk = np.ascontiguousarray(Wqkv[:, C + qcols])
        wv = np.ascontiguousarray(Wqkv[:, 2 * C + qcols])
        wo = np.stack([Wo[hA * HD : (hA + 1) * HD], Wo[hB * HD : (hB + 1) * HD]], 1)
        in_maps.append(
            {
                "xT": xT,
                "wq": wq,
                "wk": wk,
                "wv": wv,
                "wo": np.ascontiguousarray(wo),
            }
        )

    res = run_bass_kernel_spmd(nc, in_maps, core_ids=list(range(NCORES)))
    global _LAST_RES
    _LAST_RES = res

    out = np.empty((VP, B, C, H, W), dtype=np.float32)
    for b in range(B):
        acc = np.zeros((C, T), dtype=np.float32)
        for d in range(b * 4, b * 4 + 4):
            acc += res.results[d]["partialT"]
        acc += bo[:, None]
        out[:, b] = acc.reshape(C, VP, H, W).transpose(1, 0, 2, 3)
    return out


# revision 21
# speedup vs baseline: 1.3204x; 1.3204x over previous
"""Block-causal attention kernel for trn2, sharded over 8 NeuronCores.

Sharding: device d handles batch b = d // 4 and heads hA = 2*(d%4),
hB = hA + 1.  Each device computes its two heads' attention plus its
partial output projection partialT[c, t] = sum_h yT_h @ Wo_h; the host
sums the 4 partials per batch and adds bo.

All matmuls run in float32r (TF32-like, full PE rate at N>=256).
Softmax is computed without max-subtraction (scores are ~N(0,1); exp is
safe in fp32) with the denominator produced by a ones-column in the AV
stationary operand.
"""

import json

import numpy as np

import concourse.bass as bass
import concourse.mybir as mybir
import concourse.tile as tile
from concourse.bass_utils import run_bass_kernel_spmd
from concourse.masks import make_identity
from concourse.vector_clock import ScopedClock

F32R = mybir.dt.float32r
F32 = mybir.dt.float32

VP, B, C, H, W = 8, 2, 512, 16, 16
NH = 8
HD = C // NH  # 64
HWD = H * W  # 256 = block size
T = VP * HWD  # 2048
NCORES = 8
HEADS_PER_CORE = 2
SCALE = 1.0 / np.sqrt(HD)

# ---------------------------------------------------------------------------
# Container workarounds (walrus in this image rejects >1 sync wait/update per
# instruction; Tile's tail drain carries many).
# ---------------------------------------------------------------------------


def _split_syncs(bir_bytes: bytes) -> bytes:
    j = json.loads(bir_bytes)
    changed = False
    for fn in j.get("functions", []):
        for bb in fn.get("blocks", []):
            out = []
            for inst in bb.get("instructions", []):
                si = inst.get("sync_info")
                if not si:
                    out.append(inst)
                    continue
                waits = si.get("on_wait") or []
                upds = si.get("on_update") or []
                if len(waits) > 1:
                    for i, w in enumerate(waits[:-1]):
                        out.append(
                            {
                                "debug": inst.get("debug", 0),
                                "engine": inst["engine"],
                                "ins": [],
                                "name": f"{inst['name']}_sw{i}",
                                "opcode": "EventSemaphore",
                                "outs": [],
                                "sync_info": {"on_update": [], "on_wait": [w]},
                            }
                        )
                    si["on_wait"] = waits[-1:]
                    changed = True
                out.append(inst)
                if len(upds) > 1:
                    si["on_update"] = upds[:1]
                    for i, u in enumerate(upds[1:]):
                        out.append(
                            {
                                "debug": inst.get("debug", 0),
                                "engine": inst["engine"],
                                "ins": [],
                                "name": f"{inst['name']}_su{i}",
                                "opcode": "EventSemaphore",
                                "outs": [],
                                "sync_info": {"on_update": [u], "on_wait": []},
                            }
                        )
                    changed = True
            bb["instructions"] = out
    return json.dumps(j).encode() if changed else bir_bytes


_patched = False


def _install_patches():
    global _patched
    if _patched:
        return
    _patched = True

    import concourse.bass2jax as bass2jax
    from concourse.bass_utils import compile_bir_kernel as _real_compile

    def patched_compile(bir_json, tmpdir, neff_name="file.neff"):
        return _real_compile(_split_syncs(bir_json), tmpdir, neff_name=neff_name)

    bass2jax.compile_bir_kernel = patched_compile

    def _drain_and_barrier(self, tick_clock, wait_clock):
        nc = self.nc
        drain_inst = nc.sync.drain()
        wait_clock.add_sem_waits(
            drain_inst.ins, ScopedClock({None: tick_clock.global_clock})
        )
        si = drain_inst.ins.sync_info
        waits = list(si.on_wait or [])
        if len(waits) > 1:
            si.on_wait = waits[:1]
            for w in waits[1:]:
                d2 = nc.sync.drain()
                d2.ins.sync_info = mybir.SyncInfo(on_wait=[w], on_update=[])
        nc.all_engine_barrier()
        assert self.sems is not None
        popped = nc._tile_sem_poison_stack.pop()
        assert popped is self._sem_poison
        nc.clear_and_free_semaphores(list(self.sems.allocated().values()))
        nc.all_engine_barrier()

    tile.TileContext._drain_and_barrier = _drain_and_barrier


# ---------------------------------------------------------------------------
# Device program (SPMD — same program on all 8 cores, different data)
# ---------------------------------------------------------------------------


def _build_program():
    _install_patches()
    nc = bass.Bass("TRN2", target_bir_lowering=False, debug=False, num_devices=NCORES)

    xT = nc.dram_tensor("xT", [C, T], F32R, kind="ExternalInput")
    wq = nc.dram_tensor("wq", [C, 128], F32R, kind="ExternalInput")
    wk = nc.dram_tensor("wk", [C, 128], F32R, kind="ExternalInput")
    wv = nc.dram_tensor("wv", [C, 128], F32R, kind="ExternalInput")
    # wo[0:64, s, :] = Wo rows of head A (s=0) / head B (s=1)
    wo = nc.dram_tensor("wo", [64, 2, C], F32R, kind="ExternalInput")
    partialT = nc.dram_tensor("partialT", [C, T], F32, kind="ExternalOutput")

    import os

    debug = bool(os.environ.get("KDEBUG"))
    dbg = {}
    if debug:
        dbg["qT"] = nc.dram_tensor("dbg_qT", [128, T], F32, kind="ExternalOutput")
        dbg["kT"] = nc.dram_tensor("dbg_kT", [128, T], F32, kind="ExternalOutput")
        dbg["vA"] = nc.dram_tensor("dbg_vA", [128, 16, 65], F32, kind="ExternalOutput")
        dbg["pA1"] = nc.dram_tensor("dbg_pA1", [128, 1024], F32, kind="ExternalOutput")
        dbg["yA"] = nc.dram_tensor("dbg_yA", [65, 2048], F32, kind="ExternalOutput")
        dbg["rbcA"] = nc.dram_tensor("dbg_rbcA", [64, 2048], F32, kind="ExternalOutput")
        dbg["yTA"] = nc.dram_tensor("dbg_yTA", [64, T], F32, kind="ExternalOutput")

    NB = T // 256  # 8 blocks of 256 (= one VP slice each)

    with tile.TileContext(nc) as tc:
        with (
            tc.tile_pool(name="persist", bufs=1) as pers,
            tc.tile_pool(name="work", bufs=3) as work,
            tc.tile_pool(name="ppool", bufs=3) as ppool,
            tc.tile_pool(name="ypool", bufs=2) as ypool,
            tc.tile_pool(name="dpool", bufs=2, space="DRAM") as dpool,
        ):
            # ---- persistent SBUF tiles
            qT_t = pers.tile([128, T], F32R)  # rows 0-63 head A, 64-127 head B
            kT_t = pers.tile([128, T], F32R)
            vA_t = pers.tile([128, NB * 2, 65], F32R)  # [k-tok, chunk, v|1]
            vB_t = pers.tile([128, NB * 2, 65], F32R)
            yTA_t = pers.tile([64, T], F32R)
            yTB_t = pers.tile([64, T], F32R)
            wq_t = pers.tile([128, 4, 128], F32R)
            wk_t = pers.tile([128, 4, 128], F32R)
            wv_t = pers.tile([128, 4, 128], F32R)
            wo_t = pers.tile([64, 2, C], F32R)
            ident = pers.tile([128, 128], F32)
            ones_t = pers.tile([128, 1], F32)
            onesr_t = pers.tile([128, 64], F32R)
            xT_t = pers.tile([128, 4, T], F32R)

            nc.gpsimd.dma_start(out=wq_t[:], in_=wq.rearrange("(c p) m -> p c m", p=128))
            nc.gpsimd.dma_start(out=wk_t[:], in_=wk.rearrange("(c p) m -> p c m", p=128))
            nc.gpsimd.dma_start(out=wv_t[:], in_=wv.rearrange("(c p) m -> p c m", p=128))
            nc.gpsimd.dma_start(out=wo_t[:], in_=wo[:])
            make_identity(nc, ident)
            nc.vector.memset(ones_t[:], 1.0)
            nc.vector.tensor_copy(onesr_t[:], ones_t[:].to_broadcast([128, 64]))
            # preload the exp table set while the x DMA streams in
            warm = work.tile([128, 1], F32, tag="warm")
            nc.scalar.activation(
                warm[:], ones_t[:], mybir.ActivationFunctionType.Exp
            )
            for cc in range(4):
                for hh in range(4):
                    hsl = slice(hh * (T // 4), (hh + 1) * (T // 4))
                    eng = nc.sync if (cc * 4 + hh) % 2 == 0 else nc.gpsimd
                    eng.dma_start(
                        out=xT_t[:, cc, hsl], in_=xT[cc * 128 : (cc + 1) * 128, hsl]
                    )

            # ones columns of v' (free-dim broadcast copy, rounds to f32r)
            nc.vector.tensor_copy(
                vA_t[:, :, 64:65], ones_t[:].to_broadcast([128, NB * 2, 1])
            )
            nc.vector.tensor_copy(
                vB_t[:, :, 64:65], ones_t[:].to_broadcast([128, NB * 2, 1])
            )

            # ---- phase 1: qkv projections
            with tc.tile_pool(name="ps1", bufs=2, space="PSUM") as ps1:
                for sp in range(4):  # 512-wide token spans
                    sl = slice(sp * 512, (sp + 1) * 512)
                    psq = ps1.tile([128, 512], F32, tag="q")
                    psk = ps1.tile([128, 512], F32, tag="k")
                    for cc in range(4):
                        nc.tensor.matmul(
                            psq[:],
                            wq_t[:, cc, :],
                            xT_t[:, cc, sl],
                            start=(cc == 0),
                            stop=(cc == 3),
                        )
                    for cc in range(4):
                        nc.tensor.matmul(
                            psk[:],
                            wk_t[:, cc, :],
                            xT_t[:, cc, sl],
                            start=(cc == 0),
                            stop=(cc == 3),
                        )
                    nc.vector.tensor_copy(qT_t[:, sl], psq[:])
                    nc.vector.tensor_copy(kT_t[:, sl], psk[:])

                for sp in range(4):
                    sl = slice(sp * 512, (sp + 1) * 512)
                    psv = ps1.tile([128, 512], F32, tag="v")
                    for cc in range(4):
                        nc.tensor.matmul(
                            psv[:],
                            wv_t[:, cc, :],
                            xT_t[:, cc, sl],
                            start=(cc == 0),
                            stop=(cc == 3),
                        )
                    vT_sb = work.tile([128, 512], F32, tag="vT")
                    nc.vector.tensor_copy(vT_sb[:], psv[:])
                    for i in range(4):
                        j = sp * 4 + i  # k-chunk index (128 tokens)
                        trp = ps1.tile([128, 128], F32, tag="tr")
                        nc.tensor.transpose(
                            trp[:], vT_sb[:, i * 128 : (i + 1) * 128], ident[:]
                        )
                        nc.vector.tensor_copy(vA_t[:, j, 0:64], trp[:, 0:64])
                        nc.vector.tensor_copy(vB_t[:, j, 0:64], trp[:, 64:128])

            if debug:
                qT_f = work.tile([128, T], F32, tag="dbgbig")
                nc.vector.tensor_copy(qT_f[:], qT_t[:].bitcast(F32))
                nc.sync.dma_start(out=dbg["qT"][:], in_=qT_f[:])
                kT_f = work.tile([128, T], F32, tag="dbgbig")
                nc.vector.tensor_copy(kT_f[:], kT_t[:].bitcast(F32))
                nc.sync.dma_start(out=dbg["kT"][:], in_=kT_f[:])
                vA_f = work.tile([128, 16 * 65], F32, tag="dbgbig")
                nc.vector.tensor_copy(
                    vA_f[:].rearrange("p (j m) -> p j m", j=16), vA_t[:].bitcast(F32)
                )
                nc.sync.dma_start(
                    out=dbg["vA"][:],
                    in_=vA_f[:].rearrange("p (j m) -> p j m", j=16),
                )

            # ---- phase 2+3: per-span attention (q-outer) + fused projection
            def normalize_span(sp, yA_ps, yB_ps):
                """Normalize absolute 512-span sp into the yT tiles."""
                sl = slice(sp * 512, (sp + 1) * 512)
                for hi, (y_ps, yT_t) in enumerate(((yA_ps, yTA_t), (yB_ps, yTB_t))):
                    rd = work.tile([128, 512], F32, tag="rd")
                    nc.vector.reciprocal(rd[64:65, :], y_ps[64:65, :])
                    rbc = work.tile([64, 512], F32, tag="rbc")
                    dscr = dpool.tile([1, 512], F32, tag="dscr")
                    nc.sync.dma_start(out=dscr[:], in_=rd[64:65, :])
                    nc.sync.dma_start(
                        out=rbc[:], in_=dscr[0:1, :].to_broadcast([64, 512])
                    )
                    nc.vector.tensor_mul(yT_t[:, sl], y_ps[0:64, :], rbc[:])
                    if debug and hi == 0:
                        nc.sync.dma_start(out=dbg["rbcA"][:, sl], in_=rbc[:])
                        ydbg = work.tile([65, 512], F32, tag="ydbg")
                        nc.vector.tensor_copy(ydbg[:], y_ps[:, :])
                        nc.sync.dma_start(out=dbg["yA"][:, sl], in_=ydbg[:])

            def project_span(sp):
                """Output-project absolute 512-span sp (reads yT tiles)."""
                sl = slice(sp * 512, (sp + 1) * 512)
                stage = work.tile([128, 4, 512], F32, tag="po_sb")
                for cc in range(4):
                    po = pops.tile([128, 512], F32, tag="po")
                    nc.tensor.matmul(
                        po[:],
                        wo_t[:, 0, cc * 128 : (cc + 1) * 128],
                        yTA_t[:, sl],
                        start=True,
                        stop=False,
                    )
                    nc.tensor.matmul(
                        po[:],
                        wo_t[:, 1, cc * 128 : (cc + 1) * 128],
                        yTB_t[:, sl],
                        start=False,
                        stop=True,
                    )
                    nc.vector.tensor_copy(stage[:, cc, :], po[:])
                    eng = nc.sync if cc % 2 == 0 else nc.gpsimd
                    eng.dma_start(
                        out=partialT[cc * 128 : (cc + 1) * 128, sl],
                        in_=stage[:, cc, :],
                    )

            with (
                tc.tile_pool(name="stps", bufs=2, space="PSUM") as stps,
                tc.tile_pool(name="yps", bufs=1, space="PSUM") as yps,
                tc.tile_pool(name="pops", bufs=2, space="PSUM") as pops,
            ):
                for sp in range(4):
                    q0 = sp * 512
                    yA_ps = yps.tile([65, 512], F32, tag="yA")
                    yB_ps = yps.tile([65, 512], F32, tag="yB")
                    njc = 4 * sp + 4
                    for j in range(njc):
                        jb = j // 2
                        # within this span, chunk j attends q >= 256*jb
                        off = 256 if jb == 2 * sp + 1 else 0
                        Lg = 512 - off
                        ksl = slice(j * 128, (j + 1) * 128)
                        qsl = slice(q0 + off, q0 + 512)
                        stAB = stps.tile([128, 1024], F32, tag="st")
                        # heads packed tight: A at [0, Lg), B at [Lg, 2Lg)
                        nc.tensor.matmul(
                            stAB[:, 0:Lg],
                            kT_t[0:64, ksl],
                            qT_t[0:64, qsl],
                            start=True,
                            stop=True,
                        )
                        nc.tensor.matmul(
                            stAB[:, Lg : 2 * Lg],
                            kT_t[64:128, ksl],
                            qT_t[64:128, qsl],
                            start=True,
                            stop=True,
                        )
                        pAB = ppool.tile([128, 1024], F32R, tag="pAB")
                        nc.scalar.activation(
                            pAB[:, 0 : 2 * Lg],
                            stAB[:, 0 : 2 * Lg],
                            mybir.ActivationFunctionType.Exp,
                        )
                        if debug and sp == 0 and j == 1:
                            pA_f = work.tile([128, 512], F32, tag="pAf")
                            nc.vector.tensor_copy(
                                pA_f[:, 0:Lg], pAB[:, 0:Lg].bitcast(F32)
                            )
                            nc.sync.dma_start(
                                out=dbg["pA1"][:, 0:Lg], in_=pA_f[:, 0:Lg]
                            )
                        # start=True clears has_written for the whole PSUM
                        # bank: only j=0 (which covers the full bank) sets it
                        stop_j = njc - 1 if off == 256 else njc - 3
                        nc.tensor.matmul(
                            yA_ps[:, off:512],
                            vA_t[:, j, :],
                            pAB[:, 0:Lg],
                            start=(j == 0),
                            stop=(j == stop_j),
                        )
                        nc.tensor.matmul(
                            yB_ps[:, off:512],
                            vB_t[:, j, :],
                            pAB[:, Lg : 2 * Lg],
                            start=(j == 0),
                            stop=(j == stop_j),
                        )
                        if sp > 0 and j == 2:
                            project_span(sp - 1)
                    normalize_span(sp, yA_ps, yB_ps)
                project_span(3)
                if debug:
                    yTA_f = work.tile([64, T], F32, tag="dbgy")
                    nc.vector.tensor_copy(yTA_f[:], yTA_t[:].bitcast(F32))
                    nc.sync.dma_start(out=dbg["yTA"][:], in_=yTA_f[:])
    return nc


_NC_CACHE = None


def _get_program():
    global _NC_CACHE
    if _NC_CACHE is None:
        _NC_CACHE = _build_program()
    return _NC_CACHE


def kernel(x, Wqkv, bqkv, bo=None, Wo=None, **kw):
    # accept arbitrary kw order; reference signature: x, Wqkv, bqkv, Wo, bo
    if Wo is None:
        Wo = kw["Wo"]
    if bo is None:
        bo = kw["bo"]
    x = np.asarray(x, dtype=np.float32)
    Wqkv = np.asarray(Wqkv, dtype=np.float32)
    bqkv = np.asarray(bqkv, dtype=np.float32)
    Wo = np.asarray(Wo, dtype=np.float32)
    bo = np.asarray(bo, dtype=np.float32)
    assert np.all(bqkv == 0.0), "nonzero bqkv not supported by this kernel build"

    nc = _get_program()
    in_maps = []
    for d in range(NCORES):
        b = d // 4
        hA = HEADS_PER_CORE * (d % 4)
        hB = hA + 1
        # xT [C, T]: t = (v, h, w)
        xT = np.ascontiguousarray(
            x[:, b].transpose(1, 0, 2, 3).reshape(C, T)
        )
        qcols = np.r_[hA * HD : (hA + 1) * HD, hB * HD : (hB + 1) * HD]
        wq = np.ascontiguousarray(Wqkv[:, qcols] * SCALE)
        wk = np.ascontiguousarray(Wqkv[:, C + qcols])
        wv = np.ascontiguousarray(Wqkv[:, 2 * C + qcols])
        wo = np.stack([Wo[hA * HD : (hA + 1) * HD], Wo[hB * HD : (hB + 1) * HD]], 1)
        in_maps.append(
            {
                "xT": xT,
                "wq": wq,
                "wk": wk,
                "wv": wv,
                "wo": np.ascontiguousarray(wo),
            }
        )

    res = run_bass_kernel_spmd(nc, in_maps, core_ids=list(range(NCORES)))
    global _LAST_RES
    _LAST_RES = res

    out = np.empty((VP, B, C, H, W), dtype=np.float32)
    for b in range(B):
        acc = np.zeros((C, T), dtype=np.float32)
        for d in range(b * 4, b * 4 + 4):
            acc += res.results[d]["partialT"]
        acc += bo[:, None]
        out[:, b] = acc.reshape(C, VP, H, W).transpose(1, 0, 2, 3)
    return out
